# revision 2
# baseline (speedup 1.0000x reference)
"""Trainium2 Bass kernel for nn_AutoSparseLinear.

Problem: out[b,h,o] = sum_d gathered[b,h,d] * W[h,o,d] + bias[h,o]
  where gathered[b,h,k*64+w] = x[b, mask[h,k], w]
  x: [512,128,64] f32, mask: [256,4] i64, W: [256,64,256] f32, b: [256,64] f32
  out: [512,256,64] f32

Strategy (expert-style sharding per the hint): split the H_out group dim
8 ways; each core computes 32 groups over the full batch B=512.

The host (this function) does the mask-dependent gather + layout prep in
numpy, producing per-core packed operands so the device program is
identical on all 8 cores (single SPMD NEFF, no mask-dependence in the
program):
  gx  [128, 32*2*512] fp16 — per (group, d-chunk) gathered-and-transposed
       x blocks: slot(h',c)[p, b] = x[b, mask[h, 2c + p//64], p%64]
  wt  [128, 32*2*64]  fp16 — per-chunk transposed weights:
       slot(h',c)[p, o] = W[h, o, c*128+p]
  bb  [128, 16] f32 — bias pairs: col j = concat(b[2j], b[2j+1])

Device per group-pair j (groups 2j, 2j+1 side by side in PE columns):
  psum[0:64, :]   = wt(2j,0).T   @ gx(2j,0)   + wt(2j,1).T   @ gx(2j,1)
  psum[64:128, :] = wt(2j+1,0).T @ gx(2j+1,0) + wt(2j+1,1).T @ gx(2j+1,1)
  out_sb = psum + bias_col (DVE), staged 4 pairs wide, DMA'd to DRAM as
  [128, 4*512] chunks into out [128, 16*512].

Compute is fp16 (inputs rounded to fp16, fp32 PSUM accumulation);
outputs stored fp16 and upcast on host.
"""

import numpy as np

import concourse.mybir as mybir
from concourse import bacc
from concourse.tile import TileContext
from concourse.bass_utils import run_bass_kernel_spmd

# Problem shapes (hardcoded per contract)
B = 512
H_IN = 128
W_IN = 64
H_OUT = 256
W_OUT = 64
K = 4
D = K * W_IN  # 256
N_CORES = 8
HG = H_OUT // N_CORES  # 32 groups per core
N_PAIRS = HG // 2  # 16
N_SLICES = 2  # gx upload pipelining granularity
PAIRS_PER_SLICE = N_PAIRS // N_SLICES
OUT_SLICES = 4  # output DMA granularity (pairs per out chunk = 4)
PAIRS_PER_OUT = N_PAIRS // OUT_SLICES

F16 = mybir.dt.float16
F32 = mybir.dt.float32


def build_nc(
    loop: int = 1,
    mode: str = "full",
    in_dma: str = "gpsimd",
    out_dma: str = "sync",
    timing: bool = False,
):
    """Build the (uniform-across-cores) Bass program.

    loop > 1 wraps the body in a hardware For_i loop — used only for
    steady-state timing.  mode: "full" | "upload" (DMAs only) |
    "compute" (uploads hoisted out of the loop).
    """
    nc = bacc.Bacc(None, target_bir_lowering=False)
    in_dmae = getattr(nc, in_dma)
    out_dmae = getattr(nc, out_dma)
    gx_d = nc.dram_tensor("gx", [128, HG * 2 * B], F16, kind="ExternalInput")
    wt_d = nc.dram_tensor("wt", [128, HG * 2 * W_OUT], F16, kind="ExternalInput")
    bb_d = nc.dram_tensor("bb", [128, N_PAIRS], F32, kind="ExternalInput")
    if timing:
        # Keep HBM out-traffic but avoid shipping 2MB/core back over the
        # axon tunnel per bench call: write to Internal DRAM, expose a
        # tiny sink as the only ExternalOutput.
        out_d = nc.dram_tensor("out", [128, N_PAIRS * B], F16)
        sink_d = nc.dram_tensor("sink", [128, 1], F16, kind="ExternalOutput")
    else:
        out_d = nc.dram_tensor("out", [128, N_PAIRS * B], F16, kind="ExternalOutput")
        sink_d = None

    gx_cols = PAIRS_PER_SLICE * 2 * 2 * B  # per-slice gx columns

    with TileContext(nc) as tc:
        with (
            tc.tile_pool(name="res", bufs=1) as res,
            tc.tile_pool(name="psum", bufs=8, space="PSUM") as psump,
            tc.tile_pool(name="outs", bufs=2) as outp,
        ):

            def uploads():
                bt = res.tile([128, N_PAIRS], F32, tag="bias")
                in_dmae.dma_start(out=bt[:], in_=bb_d[:, :])
                wtile = res.tile([128, HG * 2 * W_OUT], F16, tag="wt")
                in_dmae.dma_start(out=wtile[:], in_=wt_d[:, :])
                gxs = []
                for s in range(N_SLICES):
                    gtile = res.tile([128, gx_cols], F16, tag=f"gx{s}")
                    in_dmae.dma_start(
                        out=gtile[:], in_=gx_d[:, s * gx_cols : (s + 1) * gx_cols]
                    )
                    gxs.append(gtile)
                return bt, wtile, gxs

            def compute(bt, wtile, gxs):
                for oc in range(OUT_SLICES):
                    ob = outp.tile([128, PAIRS_PER_OUT * B], F16, tag=f"ob{oc % 2}")
                    for jj in range(PAIRS_PER_OUT):
                        j = oc * PAIRS_PER_OUT + jj
                        s = j // PAIRS_PER_SLICE
                        ps = psump.tile([128, B], F32, tag="ps")
                        for c in range(2):
                            for hh in range(2):  # group 2j+hh -> psum rows 64*hh
                                lg = (2 * j + hh) * 2 + c
                                lhsT = wtile[:, lg * W_OUT : (lg + 1) * W_OUT]
                                lr = lg - s * (PAIRS_PER_SLICE * 4)
                                rhs = gxs[s][:, lr * B : (lr + 1) * B]
                                nc.tensor.matmul(
                                    ps[64 * hh : 64 * hh + 64, :],
                                    lhsT,
                                    rhs,
                                    start=(c == 0),
                                    stop=(c == 1),
                                )
                        nc.vector.tensor_scalar_add(
                            ob[:, jj * B : (jj + 1) * B], ps[:, :], bt[:, j : j + 1]
                        )
                    out_dmae.dma_start(
                        out=out_d[
                            :, oc * PAIRS_PER_OUT * B : (oc + 1) * PAIRS_PER_OUT * B
                        ],
                        in_=ob[:],
                    )

            def body(_iv=None):
                args = uploads()
                if mode != "upload":
                    compute(*args)

            if mode == "compute":
                args = uploads()
                if loop > 1:
                    with tc.For_i(0, loop, 1):
                        compute(*args)
                else:
                    compute(*args)
            elif loop > 1:
                with tc.For_i(0, loop, 1):
                    body()
            else:
                body()

            if sink_d is not None:
                # value is irrelevant; NEFF completion waits for all queues
                st = res.tile([128, 1], F16, tag="sinksrc")
                nc.vector.memset(st[:], 0.0)
                out_dmae.dma_start(out=sink_d[:, :], in_=st[:])

    nc.finalize()
    return nc


def shard_inputs(x, mask, W, b):
    """Host-side gather + layout prep. Returns per-core input dicts."""
    x = np.asarray(x, dtype=np.float32)
    mask = np.asarray(mask)
    W = np.asarray(W, dtype=np.float32)
    b = np.asarray(b, dtype=np.float32)

    xT = np.ascontiguousarray(x.transpose(1, 2, 0))  # [i, w, b]
    in_maps = []
    for q in range(N_CORES):
        h0 = q * HG
        mq = mask[h0 : h0 + HG]  # [HG, 4]
        g = xT[mq]  # [HG, 4, 64, B]
        g = g.reshape(HG, 2, 128, B).transpose(2, 0, 1, 3)  # [128, HG, 2, B]
        gx = np.ascontiguousarray(g.reshape(128, HG * 2 * B)).astype(
            np.float16
        )

        Wq = W[h0 : h0 + HG]  # [HG, 64, 256]
        wt = (
            Wq.transpose(0, 2, 1)  # [HG, d, o]
            .reshape(HG, 2, 128, W_OUT)
            .transpose(2, 0, 1, 3)  # [128, HG, 2, o]
            .reshape(128, HG * 2 * W_OUT)
        )
        wt = np.ascontiguousarray(wt).astype(np.float16)

        bb = np.empty((128, N_PAIRS), np.float32)
        for j in range(N_PAIRS):
            bb[:64, j] = b[h0 + 2 * j]
            bb[64:, j] = b[h0 + 2 * j + 1]

        in_maps.append({"gx": gx, "wt": wt, "bb": bb})
    return in_maps


def assemble_output(results):
    """results: list of per-core dicts with 'out' [128, N_PAIRS*B] f16."""
    out = np.empty((B, H_OUT, W_OUT), np.float32)
    for q, r in enumerate(results):
        o = np.asarray(r["out"], dtype=np.float32).reshape(2, W_OUT, N_PAIRS, B)
        # o[hh, o, j, b] -> out[b, q*HG + 2j + hh, o]
        out[:, q * HG : (q + 1) * HG, :] = (
            o.transpose(3, 2, 0, 1).reshape(B, HG, W_OUT)
        )
    return out


_NC_CACHE = {}


def kernel(x, mask, W, b):
    in_maps = shard_inputs(x, mask, W, b)
    if "nc" not in _NC_CACHE:
        _NC_CACHE["nc"] = build_nc()
    nc = _NC_CACHE["nc"]
    res = run_bass_kernel_spmd(nc, in_maps, core_ids=list(range(N_CORES)))
    return assemble_output(res.results)


# revision 5
# speedup vs baseline: 1.5228x; 1.5228x over previous
"""Trainium2 Bass kernel for nn_AutoSparseLinear.

Problem: out[b,h,o] = sum_d gathered[b,h,d] * W[h,o,d] + bias[h,o]
  where gathered[b,h,k*64+w] = x[b, mask[h,k], w]
  x: [512,128,64] f32, mask: [256,4] i64, W: [256,64,256] f32, b: [256,64] f32
  out: [512,256,64] f32

Strategy (expert-style sharding per the hint): split the H_out group dim
8 ways; each core computes 32 groups over the full batch B=512.

The kernel is HBM-bandwidth bound (~290 GB/s/core under 8-way load), so
inputs ship as int8 and are cast to f16 by the DMA engines in flight:
  - x rows are quantized per-row: xq = round(x / sx[i]), sx per H_in row.
  - W blocks absorb sx: W''(h,k) = W[h,:,k*64:+64] * sx[mask[h,k]], then
    quantized per (h,o): Wq = round(W'' / sw[h,o]).
  - psum therefore holds integer-valued products; the drain applies
    out = psum * sw + bias in one fused DVE tensor_scalar (mult, add).

Host-prepped per-core operands (uniform SPMD program, mask-dependence
lives in the data only):
  gx  [128, 32*2*512] int8 — per (group, d-chunk) gathered x blocks:
       slot(h',c)[p, b] = xq[b, mask[h, 2c + p//64], p%64]
  wt  [128, 32*2*64]  int8 — per-chunk transposed quantized weights
  scb [128, 32] f32 — col j: drain scale pair j; col 16+j: bias pair j

Device per group-pair j (groups 2j, 2j+1 side by side in PE columns):
  psum[0:64, :]   = wt(2j,0).T   @ gx(2j,0)   + wt(2j,1).T   @ gx(2j,1)
  psum[64:128, :] = wt(2j+1,0).T @ gx(2j+1,0) + wt(2j+1,1).T @ gx(2j+1,1)
  ob = psum * sw + bias (DVE), staged 4 pairs wide, DMA'd to DRAM as
  [128, 4*512] f16 chunks into out [128, 16*512].
"""

import numpy as np

import concourse.mybir as mybir
from concourse import bacc
from concourse.tile import TileContext
from concourse.bass_utils import run_bass_kernel_spmd

# Problem shapes (hardcoded per contract)
B = 512
H_IN = 128
W_IN = 64
H_OUT = 256
W_OUT = 64
K = 4
N_CORES = 8
HG = H_OUT // N_CORES  # 32 groups per core
N_PAIRS = HG // 2  # 16
N_SLICES = 2  # gx upload pipelining granularity
OUT_SLICES = 4  # output DMA granularity

F16 = mybir.dt.float16
F32 = mybir.dt.float32
I8 = mybir.dt.int8


def build_nc(
    loop: int = 1,
    mode: str = "full",
    out_dma: str = "gpsimd",
    n_slices: int = N_SLICES,
    out_slices: int = OUT_SLICES,
    timing: bool = False,
):
    """Build the (uniform-across-cores) Bass program.

    loop > 1 wraps the body in a hardware For_i loop — used only for
    steady-state timing.  mode: "full" | "upload" (DMAs only) |
    "compute" (uploads hoisted out of the loop).
    """
    nc = bacc.Bacc(None, target_bir_lowering=False)
    in_dmae = nc.gpsimd  # SWDGE required: these DMAs cast int8 -> f16
    out_dmae = getattr(nc, out_dma)
    gx_d = nc.dram_tensor("gx", [128, HG * 2 * B], I8, kind="ExternalInput")
    wt_d = nc.dram_tensor("wt", [128, HG * 2 * W_OUT], I8, kind="ExternalInput")
    scb_d = nc.dram_tensor("scb", [128, 2 * N_PAIRS], F32, kind="ExternalInput")
    if timing:
        # Keep HBM out-traffic but avoid shipping 2MB/core back over the
        # axon tunnel per bench call: write to Internal DRAM, expose a
        # tiny sink as the only ExternalOutput.
        out_d = nc.dram_tensor("out", [128, N_PAIRS * B], F16)
        sink_d = nc.dram_tensor("sink", [128, 1], F16, kind="ExternalOutput")
    else:
        out_d = nc.dram_tensor("out", [128, N_PAIRS * B], F16, kind="ExternalOutput")
        sink_d = None

    pairs_per_slice = N_PAIRS // n_slices
    pairs_per_out = N_PAIRS // out_slices
    gx_cols = pairs_per_slice * 2 * 2 * B  # per-slice gx columns

    with TileContext(nc) as tc:
        with (
            tc.tile_pool(name="res", bufs=1) as res,
            tc.tile_pool(name="psum", bufs=8, space="PSUM") as psump,
            tc.tile_pool(name="outs", bufs=2) as outp,
        ):

            def uploads():
                st = res.tile([128, 2 * N_PAIRS], F32, tag="scb")
                in_dmae.dma_start(out=st[:], in_=scb_d[:, :])
                wtile = res.tile([128, HG * 2 * W_OUT], F16, tag="wt")
                in_dmae.dma_start(out=wtile[:], in_=wt_d[:, :])
                gxs = []
                for s in range(n_slices):
                    gtile = res.tile([128, gx_cols], F16, tag=f"gx{s}")
                    in_dmae.dma_start(
                        out=gtile[:], in_=gx_d[:, s * gx_cols : (s + 1) * gx_cols]
                    )
                    gxs.append(gtile)
                return st, wtile, gxs

            def compute(st, wtile, gxs):
                for oc in range(out_slices):
                    ob = outp.tile([128, pairs_per_out * B], F16, tag=f"ob{oc % 2}")
                    for jj in range(pairs_per_out):
                        j = oc * pairs_per_out + jj
                        s = j // pairs_per_slice
                        ps = psump.tile([128, B], F32, tag="ps")
                        for c in range(2):
                            for hh in range(2):  # group 2j+hh -> psum rows 64*hh
                                lg = (2 * j + hh) * 2 + c
                                lhsT = wtile[:, lg * W_OUT : (lg + 1) * W_OUT]
                                lr = lg - s * (pairs_per_slice * 4)
                                rhs = gxs[s][:, lr * B : (lr + 1) * B]
                                nc.tensor.matmul(
                                    ps[64 * hh : 64 * hh + 64, :],
                                    lhsT,
                                    rhs,
                                    start=(c == 0),
                                    stop=(c == 1),
                                )
                        nc.vector.tensor_scalar(
                            ob[:, jj * B : (jj + 1) * B],
                            ps[:, :],
                            st[:, j : j + 1],
                            st[:, N_PAIRS + j : N_PAIRS + j + 1],
                            mybir.AluOpType.mult,
                            mybir.AluOpType.add,
                        )
                    out_dmae.dma_start(
                        out=out_d[
                            :, oc * pairs_per_out * B : (oc + 1) * pairs_per_out * B
                        ],
                        in_=ob[:],
                    )

            def body(_iv=None):
                args = uploads()
                if mode != "upload":
                    compute(*args)

            if mode == "compute":
                args = uploads()
                if loop > 1:
                    with tc.For_i(0, loop, 1):
                        compute(*args)
                else:
                    compute(*args)
            elif loop > 1:
                with tc.For_i(0, loop, 1):
                    body()
            else:
                body()

            if sink_d is not None:
                # value is irrelevant; NEFF completion waits for all queues
                st2 = res.tile([128, 1], F16, tag="sinksrc")
                nc.vector.memset(st2[:], 0.0)
                out_dmae.dma_start(out=sink_d[:, :], in_=st2[:])

    nc.finalize()
    return nc


def shard_inputs(x, mask, W, b):
    """Host-side quantize + gather + layout prep. Returns per-core inputs."""
    x = np.asarray(x, dtype=np.float32)
    mask = np.asarray(mask)
    W = np.asarray(W, dtype=np.float32)
    b = np.asarray(b, dtype=np.float32)

    # per-H_in-row int8 quantization of x
    sx = np.abs(x).max(axis=(0, 2)) / 127.0  # [H_IN]
    sx = np.maximum(sx, 1e-30)
    xq = np.clip(np.round(x / sx[None, :, None]), -127, 127).astype(np.int8)
    xqT = np.ascontiguousarray(xq.transpose(1, 2, 0))  # [i, w, b] int8

    in_maps = []
    for q in range(N_CORES):
        h0 = q * HG
        mq = mask[h0 : h0 + HG]  # [HG, 4]
        g = xqT[mq]  # [HG, 4, 64, B] int8
        g = g.reshape(HG, 2, 128, B).transpose(2, 0, 1, 3)  # [128, HG, 2, B]
        gx = np.ascontiguousarray(g.reshape(128, HG * 2 * B))

        # W'' with folded sx, then per-(h,o) int8 quantization
        Wf = W[h0 : h0 + HG].reshape(HG, W_OUT, K, W_IN)  # [h', o, k, w]
        Wpp = Wf * sx[mq][:, None, :, None]
        sw = np.abs(Wpp).max(axis=(2, 3)) / 127.0  # [h', o]
        sw = np.maximum(sw, 1e-30)
        Wq = np.clip(np.round(Wpp / sw[:, :, None, None]), -127, 127)
        wt = (
            Wq.reshape(HG, W_OUT, K * W_IN)
            .transpose(0, 2, 1)  # [h', d, o]
            .reshape(HG, 2, 128, W_OUT)
            .transpose(2, 0, 1, 3)  # [128, h', c, o]
            .reshape(128, HG * 2 * W_OUT)
        )
        wt = np.ascontiguousarray(wt).astype(np.int8)

        scb = np.empty((128, 2 * N_PAIRS), np.float32)
        for j in range(N_PAIRS):
            scb[:64, j] = sw[2 * j]
            scb[64:, j] = sw[2 * j + 1]
            scb[:64, N_PAIRS + j] = b[h0 + 2 * j]
            scb[64:, N_PAIRS + j] = b[h0 + 2 * j + 1]

        in_maps.append({"gx": gx, "wt": wt, "scb": scb})
    return in_maps


def assemble_output(results):
    """results: list of per-core dicts with 'out' [128, N_PAIRS*B] f16."""
    out = np.empty((B, H_OUT, W_OUT), np.float32)
    for q, r in enumerate(results):
        o = np.asarray(r["out"], dtype=np.float32).reshape(2, W_OUT, N_PAIRS, B)
        # o[hh, o, j, b] -> out[b, q*HG + 2j + hh, o]
        out[:, q * HG : (q + 1) * HG, :] = (
            o.transpose(3, 2, 0, 1).reshape(B, HG, W_OUT)
        )
    return out


_NC_CACHE = {}


def kernel(x, mask, W, b):
    in_maps = shard_inputs(x, mask, W, b)
    if "nc" not in _NC_CACHE:
        _NC_CACHE["nc"] = build_nc()
    nc = _NC_CACHE["nc"]
    res = run_bass_kernel_spmd(nc, in_maps, core_ids=list(range(N_CORES)))
    return assemble_output(res.results)


# revision 6
# speedup vs baseline: 1.9246x; 1.2639x over previous
"""Trainium2 Bass kernel for nn_AutoSparseLinear.

Problem: out[b,h,o] = sum_d gathered[b,h,d] * W[h,o,d] + bias[h,o]
  where gathered[b,h,k*64+w] = x[b, mask[h,k], w]
  x: [512,128,64] f32, mask: [256,4] i64, W: [256,64,256] f32, b: [256,64] f32
  out: [512,256,64] f32

Strategy (expert-style sharding per the hint): split the H_out group dim
8 ways; each core computes 32 groups over the full batch B=512.

The kernel is DMA-bandwidth bound (~290 GB/s/core HBM, ~340 GB/s/core
SBUF-fabric under 8-way load), so input bytes are minimized:
  - gathered x ships as fp8 E3M4 (raw bytes in an int8 tensor; the SBUF
    tile is bitcast to float8e3 and fed to the PE directly — TensorE
    supports a mixed f16(lhsT) x f8e3(rhs) matmul at full rate).
    x is pre-scaled by 2 (max |2x| ~ 8.6 < e3m4 max 15.75) to lift small
    values out of the subnormal range; the 1/2 is folded into the drain
    scale.
  - W ships as int8, quantized per (h,o) output column, and is cast
    int8 -> f16 in flight by the DMA engines (SWDGE cast).
  - The drain applies out = psum * (sw/2) + bias in one fused DVE
    tensor_scalar (mult, add), producing f16 outputs.
Max rel error vs the f32 reference: ~1.3e-2 (threshold 2e-2).

Host-prepped per-core operands (uniform SPMD program, mask-dependence
lives in the data only):
  gx  [128, 32*2*512] int8 (e3m4 bytes) — slot(h',c)[p, b] = e3m4(2 *
       x[b, mask[h, 2c + p//64], p%64])
  wt  [128, 32*2*64]  int8 — per-chunk transposed quantized weights
  scb [128, 32] f32 — col j: drain scale pair j; col 16+j: bias pair j

Device per group-pair j (groups 2j, 2j+1 side by side in PE columns):
  psum[0:64, :]   = wt(2j,0).T   @ gx(2j,0)   + wt(2j,1).T   @ gx(2j,1)
  psum[64:128, :] = wt(2j+1,0).T @ gx(2j+1,0) + wt(2j+1,1).T @ gx(2j+1,1)
  ob = psum * sw + bias (DVE), staged 4 pairs wide, DMA'd to DRAM as
  [128, 4*512] f16 chunks into out [128, 16*512].
"""

import numpy as np
import ml_dtypes

import concourse.mybir as mybir
from concourse import bacc
from concourse.tile import TileContext
from concourse.bass_utils import run_bass_kernel_spmd

# Problem shapes (hardcoded per contract)
B = 512
H_IN = 128
W_IN = 64
H_OUT = 256
W_OUT = 64
K = 4
N_CORES = 8
HG = H_OUT // N_CORES  # 32 groups per core
N_PAIRS = HG // 2  # 16
N_SLICES = 2  # gx upload pipelining granularity
OUT_SLICES = 4  # output DMA granularity
X_SCALE = 2.0  # pre-scale for e3m4 (folded into drain scale)

F16 = mybir.dt.float16
F32 = mybir.dt.float32
I8 = mybir.dt.int8
F8E3 = mybir.dt.float8e3


def build_nc(
    loop: int = 1,
    mode: str = "full",
    out_dma: str = "gpsimd",
    n_slices: int = N_SLICES,
    out_slices: int = OUT_SLICES,
    timing: bool = False,
):
    """Build the (uniform-across-cores) Bass program.

    loop > 1 wraps the body in a hardware For_i loop — used only for
    steady-state timing.  mode: "full" | "upload" (DMAs only) |
    "compute" (uploads hoisted out of the loop).
    """
    nc = bacc.Bacc(None, target_bir_lowering=False)
    in_dmae = nc.gpsimd  # SWDGE required: the wt DMA casts int8 -> f16
    out_dmae = getattr(nc, out_dma)
    gx_d = nc.dram_tensor("gx", [128, HG * 2 * B], I8, kind="ExternalInput")
    wt_d = nc.dram_tensor("wt", [128, HG * 2 * W_OUT], I8, kind="ExternalInput")
    scb_d = nc.dram_tensor("scb", [128, 2 * N_PAIRS], F32, kind="ExternalInput")
    if timing:
        # Keep HBM out-traffic but avoid shipping 2MB/core back over the
        # axon tunnel per bench call: write to Internal DRAM, expose a
        # tiny sink as the only ExternalOutput.
        out_d = nc.dram_tensor("out", [128, N_PAIRS * B], F16)
        sink_d = nc.dram_tensor("sink", [128, 1], F16, kind="ExternalOutput")
    else:
        out_d = nc.dram_tensor("out", [128, N_PAIRS * B], F16, kind="ExternalOutput")
        sink_d = None

    pairs_per_slice = N_PAIRS // n_slices
    pairs_per_out = N_PAIRS // out_slices
    gx_cols = pairs_per_slice * 2 * 2 * B  # per-slice gx columns

    with TileContext(nc) as tc:
        with (
            tc.tile_pool(name="res", bufs=1) as res,
            tc.tile_pool(name="psum", bufs=8, space="PSUM") as psump,
            tc.tile_pool(name="outs", bufs=2) as outp,
        ):

            def uploads():
                st = res.tile([128, 2 * N_PAIRS], F32, tag="scb")
                in_dmae.dma_start(out=st[:], in_=scb_d[:, :])
                wtile = res.tile([128, HG * 2 * W_OUT], F16, tag="wt")
                in_dmae.dma_start(out=wtile[:], in_=wt_d[:, :])  # int8 -> f16
                gxs = []
                for s in range(n_slices):
                    gtile = res.tile([128, gx_cols], I8, tag=f"gx{s}")
                    in_dmae.dma_start(
                        out=gtile[:], in_=gx_d[:, s * gx_cols : (s + 1) * gx_cols]
                    )
                    gxs.append(gtile)
                return st, wtile, gxs

            def compute(st, wtile, gxs):
                for oc in range(out_slices):
                    ob = outp.tile([128, pairs_per_out * B], F16, tag=f"ob{oc % 2}")
                    for jj in range(pairs_per_out):
                        j = oc * pairs_per_out + jj
                        s = j // pairs_per_slice
                        ps = psump.tile([128, B], F32, tag="ps")
                        for c in range(2):
                            for hh in range(2):  # group 2j+hh -> psum rows 64*hh
                                lg = (2 * j + hh) * 2 + c
                                lhsT = wtile[:, lg * W_OUT : (lg + 1) * W_OUT]
                                lr = lg - s * (pairs_per_slice * 4)
                                rhs = gxs[s][:, lr * B : (lr + 1) * B].bitcast(F8E3)
                                nc.tensor.matmul(
                                    ps[64 * hh : 64 * hh + 64, :],
                                    lhsT,
                                    rhs,
                                    start=(c == 0),
                                    stop=(c == 1),
                                )
                        nc.vector.tensor_scalar(
                            ob[:, jj * B : (jj + 1) * B],
                            ps[:, :],
                            st[:, j : j + 1],
                            st[:, N_PAIRS + j : N_PAIRS + j + 1],
                            mybir.AluOpType.mult,
                            mybir.AluOpType.add,
                        )
                    out_dmae.dma_start(
                        out=out_d[
                            :, oc * pairs_per_out * B : (oc + 1) * pairs_per_out * B
                        ],
                        in_=ob[:],
                    )

            def body(_iv=None):
                args = uploads()
                if mode != "upload":
                    compute(*args)

            if mode == "compute":
                args = uploads()
                if loop > 1:
                    with tc.For_i(0, loop, 1):
                        compute(*args)
                else:
                    compute(*args)
            elif loop > 1:
                with tc.For_i(0, loop, 1):
                    body()
            else:
                body()

            if sink_d is not None:
                # value is irrelevant; NEFF completion waits for all queues
                st2 = res.tile([128, 1], F16, tag="sinksrc")
                nc.vector.memset(st2[:], 0.0)
                out_dmae.dma_start(out=sink_d[:, :], in_=st2[:])

    nc.finalize()
    return nc


def shard_inputs(x, mask, W, b):
    """Host-side quantize + gather + layout prep. Returns per-core inputs."""
    x = np.asarray(x, dtype=np.float32)
    mask = np.asarray(mask)
    W = np.asarray(W, dtype=np.float32)
    b = np.asarray(b, dtype=np.float32)

    # x -> e3m4 bytes (pre-scaled); clip to the format max to avoid inf
    xs = np.clip(x * X_SCALE, -15.5, 15.5)
    x8 = xs.astype(ml_dtypes.float8_e3m4).view(np.int8)  # [B, H_IN, W_IN]
    x8T = np.ascontiguousarray(x8.transpose(1, 2, 0))  # [i, w, b]

    in_maps = []
    for q in range(N_CORES):
        h0 = q * HG
        mq = mask[h0 : h0 + HG]  # [HG, 4]
        g = x8T[mq]  # [HG, 4, 64, B] int8(e3m4)
        g = g.reshape(HG, 2, 128, B).transpose(2, 0, 1, 3)  # [128, HG, 2, B]
        gx = np.ascontiguousarray(g.reshape(128, HG * 2 * B))

        # per-(h,o) int8 quantization of W
        Wf = W[h0 : h0 + HG].reshape(HG, W_OUT, K, W_IN)  # [h', o, k, w]
        sw = np.abs(Wf).max(axis=(2, 3)) / 127.0  # [h', o]
        sw = np.maximum(sw, 1e-30)
        Wq = np.clip(np.round(Wf / sw[:, :, None, None]), -127, 127)
        wt = (
            Wq.reshape(HG, W_OUT, K * W_IN)
            .transpose(0, 2, 1)  # [h', d, o]
            .reshape(HG, 2, 128, W_OUT)
            .transpose(2, 0, 1, 3)  # [128, h', c, o]
            .reshape(128, HG * 2 * W_OUT)
        )
        wt = np.ascontiguousarray(wt).astype(np.int8)

        scb = np.empty((128, 2 * N_PAIRS), np.float32)
        for j in range(N_PAIRS):
            scb[:64, j] = sw[2 * j] / X_SCALE
            scb[64:, j] = sw[2 * j + 1] / X_SCALE
            scb[:64, N_PAIRS + j] = b[h0 + 2 * j]
            scb[64:, N_PAIRS + j] = b[h0 + 2 * j + 1]

        in_maps.append({"gx": gx, "wt": wt, "scb": scb})
    return in_maps


def assemble_output(results):
    """results: list of per-core dicts with 'out' [128, N_PAIRS*B] f16."""
    out = np.empty((B, H_OUT, W_OUT), np.float32)
    for q, r in enumerate(results):
        o = np.asarray(r["out"], dtype=np.float32).reshape(2, W_OUT, N_PAIRS, B)
        # o[hh, o, j, b] -> out[b, q*HG + 2j + hh, o]
        out[:, q * HG : (q + 1) * HG, :] = (
            o.transpose(3, 2, 0, 1).reshape(B, HG, W_OUT)
        )
    return out


_NC_CACHE = {}


def kernel(x, mask, W, b):
    in_maps = shard_inputs(x, mask, W, b)
    if "nc" not in _NC_CACHE:
        _NC_CACHE["nc"] = build_nc()
    nc = _NC_CACHE["nc"]
    res = run_bass_kernel_spmd(nc, in_maps, core_ids=list(range(N_CORES)))
    return assemble_output(res.results)


# revision 7
# speedup vs baseline: 2.0452x; 1.0627x over previous
"""Trainium2 Bass kernel for nn_AutoSparseLinear.

Problem: out[b,h,o] = sum_d gathered[b,h,d] * W[h,o,d] + bias[h,o]
  where gathered[b,h,k*64+w] = x[b, mask[h,k], w]
  x: [512,128,64] f32, mask: [256,4] i64, W: [256,64,256] f32, b: [256,64] f32
  out: [512,256,64] f32

Strategy (expert-style sharding per the hint): split the H_out group dim
8 ways; each core computes 32 groups over the full batch B=512.

The kernel is DMA-bandwidth bound (~290 GB/s/core HBM, ~340 GB/s/core
SBUF-fabric under 8-way load), so input bytes are minimized:
  - gathered x ships as fp8 E3M4 (raw bytes in an int8 tensor; the SBUF
    tile is bitcast to float8e3 and fed to the PE directly — TensorE
    supports a mixed f16(lhsT) x f8e3(rhs) matmul at full rate).
    x is pre-scaled by 2 (max |2x| ~ 8.6 < e3m4 max 15.75) to lift small
    values out of the subnormal range; the 1/2 is folded into the drain
    scale.
  - W ships as int8, quantized per (h,o) output column, and is cast
    int8 -> f16 in flight by the DMA engines (SWDGE cast).
  - The drain applies out = psum * (sw/2) + bias in one fused DVE
    tensor_scalar (mult, add), producing f16 outputs.
Max rel error vs the f32 reference: ~1.3e-2 (threshold 2e-2).

Host-prepped per-core operands (uniform SPMD program, mask-dependence
lives in the data only):
  gx  [128, 32*2*512] int8 (e3m4 bytes) — slot(h',c)[p, b] = e3m4(2 *
       x[b, mask[h, 2c + p//64], p%64])
  wt  [128, 32*2*64]  int8 — per-chunk transposed quantized weights
  scb [128, 32] f32 — col j: drain scale pair j; col 16+j: bias pair j

Device per group-pair j (groups 2j, 2j+1 side by side in PE columns):
  psum[0:64, :]   = wt(2j,0).T   @ gx(2j,0)   + wt(2j,1).T   @ gx(2j,1)
  psum[64:128, :] = wt(2j+1,0).T @ gx(2j+1,0) + wt(2j+1,1).T @ gx(2j+1,1)
  ob = psum * sw + bias (DVE), staged 4 pairs wide, DMA'd to DRAM as
  [128, 4*512] f16 chunks into out [128, 16*512].
"""

import numpy as np
import ml_dtypes

import concourse.mybir as mybir
from concourse import bacc
from concourse.tile import TileContext
from concourse.bass_utils import run_bass_kernel_spmd

# Problem shapes (hardcoded per contract)
B = 512
H_IN = 128
W_IN = 64
H_OUT = 256
W_OUT = 64
K = 4
N_CORES = 8
HG = H_OUT // N_CORES  # 32 groups per core
N_PAIRS = HG // 2  # 16
N_SLICES = 2  # gx upload pipelining granularity
OUT_SLICES = 4  # output DMA granularity
X_SCALE = 2.0  # pre-scale for e3m4 (folded into drain scale)

F16 = mybir.dt.float16
F32 = mybir.dt.float32
I8 = mybir.dt.int8
F8E3 = mybir.dt.float8e3


def build_nc(
    loop: int = 1,
    mode: str = "full",
    out_dma: str = "gpsimd",
    n_slices: int = N_SLICES,
    out_slices: int = OUT_SLICES,
    timing: bool = False,
):
    """Build the (uniform-across-cores) Bass program.

    loop > 1 wraps the body in a hardware For_i loop — used only for
    steady-state timing.  mode: "full" | "upload" (DMAs only) |
    "compute" (uploads hoisted out of the loop).
    """
    nc = bacc.Bacc(None, target_bir_lowering=False)
    in_dmae = nc.gpsimd  # SWDGE required: the wt DMA casts int8 -> f16
    out_dmae = getattr(nc, out_dma)
    gx_d = nc.dram_tensor("gx", [128, HG * 2 * B], I8, kind="ExternalInput")
    wt_d = nc.dram_tensor("wt", [128, HG * 2 * W_OUT], I8, kind="ExternalInput")
    scb_d = nc.dram_tensor("scb", [128, 2 * N_PAIRS], F32, kind="ExternalInput")
    if timing:
        # Keep HBM out-traffic but avoid shipping 2MB/core back over the
        # axon tunnel per bench call: write to Internal DRAM, expose a
        # tiny sink as the only ExternalOutput.
        out_d = nc.dram_tensor("out", [128, N_PAIRS * B], F16)
        sink_d = nc.dram_tensor("sink", [128, 1], F16, kind="ExternalOutput")
    else:
        out_d = nc.dram_tensor("out", [128, N_PAIRS * B], F16, kind="ExternalOutput")
        sink_d = None

    pairs_per_slice = N_PAIRS // n_slices
    pairs_per_out = N_PAIRS // out_slices
    gx_cols = pairs_per_slice * 2 * 2 * B  # per-slice gx columns

    with TileContext(nc) as tc:
        with (
            tc.tile_pool(name="res", bufs=2) as res,
            tc.tile_pool(name="psum", bufs=8, space="PSUM") as psump,
            tc.tile_pool(name="outs", bufs=2) as outp,
        ):

            def uploads():
                st = res.tile([128, 2 * N_PAIRS], F32, tag="scb")
                in_dmae.dma_start(out=st[:], in_=scb_d[:, :])
                wtile = res.tile([128, HG * 2 * W_OUT], F16, tag="wt")
                in_dmae.dma_start(out=wtile[:], in_=wt_d[:, :])  # int8 -> f16
                gxs = []
                for s in range(n_slices):
                    gtile = res.tile([128, gx_cols], I8, tag=f"gx{s}")
                    in_dmae.dma_start(
                        out=gtile[:], in_=gx_d[:, s * gx_cols : (s + 1) * gx_cols]
                    )
                    gxs.append(gtile)
                return st, wtile, gxs

            def compute(st, wtile, gxs):
                for oc in range(out_slices):
                    ob = outp.tile([128, pairs_per_out * B], F16, tag=f"ob{oc % 2}")
                    for jj in range(pairs_per_out):
                        j = oc * pairs_per_out + jj
                        s = j // pairs_per_slice
                        ps = psump.tile([128, B], F32, tag="ps")
                        for c in range(2):
                            for hh in range(2):  # group 2j+hh -> psum rows 64*hh
                                lg = (2 * j + hh) * 2 + c
                                lhsT = wtile[:, lg * W_OUT : (lg + 1) * W_OUT]
                                lr = lg - s * (pairs_per_slice * 4)
                                rhs = gxs[s][:, lr * B : (lr + 1) * B].bitcast(F8E3)
                                nc.tensor.matmul(
                                    ps[64 * hh : 64 * hh + 64, :],
                                    lhsT,
                                    rhs,
                                    start=(c == 0),
                                    stop=(c == 1),
                                )
                        nc.any.tensor_scalar(
                            ob[:, jj * B : (jj + 1) * B],
                            ps[:, :],
                            st[:, j : j + 1],
                            st[:, N_PAIRS + j : N_PAIRS + j + 1],
                            mybir.AluOpType.mult,
                            mybir.AluOpType.add,
                        )
                    out_dmae.dma_start(
                        out=out_d[
                            :, oc * pairs_per_out * B : (oc + 1) * pairs_per_out * B
                        ],
                        in_=ob[:],
                    )

            def body(_iv=None):
                args = uploads()
                if mode != "upload":
                    compute(*args)

            if mode == "compute":
                args = uploads()
                if loop > 1:
                    with tc.For_i(0, loop, 1):
                        compute(*args)
                else:
                    compute(*args)
            elif loop > 1:
                with tc.For_i(0, loop, 1):
                    body()
            else:
                body()

            if sink_d is not None:
                # value is irrelevant; NEFF completion waits for all queues
                st2 = res.tile([128, 1], F16, tag="sinksrc")
                nc.vector.memset(st2[:], 0.0)
                out_dmae.dma_start(out=sink_d[:, :], in_=st2[:])

    nc.finalize()
    return nc


def shard_inputs(x, mask, W, b):
    """Host-side quantize + gather + layout prep. Returns per-core inputs."""
    x = np.asarray(x, dtype=np.float32)
    mask = np.asarray(mask)
    W = np.asarray(W, dtype=np.float32)
    b = np.asarray(b, dtype=np.float32)

    # x -> e3m4 bytes (pre-scaled); clip to the format max to avoid inf
    xs = np.clip(x * X_SCALE, -15.5, 15.5)
    x8 = xs.astype(ml_dtypes.float8_e3m4).view(np.int8)  # [B, H_IN, W_IN]
    x8T = np.ascontiguousarray(x8.transpose(1, 2, 0))  # [i, w, b]

    in_maps = []
    for q in range(N_CORES):
        h0 = q * HG
        mq = mask[h0 : h0 + HG]  # [HG, 4]
        g = x8T[mq]  # [HG, 4, 64, B] int8(e3m4)
        g = g.reshape(HG, 2, 128, B).transpose(2, 0, 1, 3)  # [128, HG, 2, B]
        gx = np.ascontiguousarray(g.reshape(128, HG * 2 * B))

        # per-(h,o) int8 quantization of W
        Wf = W[h0 : h0 + HG].reshape(HG, W_OUT, K, W_IN)  # [h', o, k, w]
        sw = np.abs(Wf).max(axis=(2, 3)) / 127.0  # [h', o]
        sw = np.maximum(sw, 1e-30)
        Wq = np.clip(np.round(Wf / sw[:, :, None, None]), -127, 127)
        wt = (
            Wq.reshape(HG, W_OUT, K * W_IN)
            .transpose(0, 2, 1)  # [h', d, o]
            .reshape(HG, 2, 128, W_OUT)
            .transpose(2, 0, 1, 3)  # [128, h', c, o]
            .reshape(128, HG * 2 * W_OUT)
        )
        wt = np.ascontiguousarray(wt).astype(np.int8)

        scb = np.empty((128, 2 * N_PAIRS), np.float32)
        for j in range(N_PAIRS):
            scb[:64, j] = sw[2 * j] / X_SCALE
            scb[64:, j] = sw[2 * j + 1] / X_SCALE
            scb[:64, N_PAIRS + j] = b[h0 + 2 * j]
            scb[64:, N_PAIRS + j] = b[h0 + 2 * j + 1]

        in_maps.append({"gx": gx, "wt": wt, "scb": scb})
    return in_maps


def assemble_output(results):
    """results: list of per-core dicts with 'out' [128, N_PAIRS*B] f16."""
    out = np.empty((B, H_OUT, W_OUT), np.float32)
    for q, r in enumerate(results):
        o = np.asarray(r["out"], dtype=np.float32).reshape(2, W_OUT, N_PAIRS, B)
        # o[hh, o, j, b] -> out[b, q*HG + 2j + hh, o]
        out[:, q * HG : (q + 1) * HG, :] = (
            o.transpose(3, 2, 0, 1).reshape(B, HG, W_OUT)
        )
    return out


_NC_CACHE = {}


def kernel(x, mask, W, b):
    in_maps = shard_inputs(x, mask, W, b)
    if "nc" not in _NC_CACHE:
        _NC_CACHE["nc"] = build_nc()
    nc = _NC_CACHE["nc"]
    res = run_bass_kernel_spmd(nc, in_maps, core_ids=list(range(N_CORES)))
    return assemble_output(res.results)
